# revision 11
# baseline (speedup 1.0000x reference)
"""Trainium2 Bass kernel for SAGAN-style spatial self-attention.

Reference computation (per batch b):
    xf = x[b].reshape(C, N)                    # C=256, N=64*64=4096
    f  = w1 @ xf                               # [32, N]   (query^T)
    g  = w2 @ xf                               # [32, N]   (key)
    V  = (w3 @ xf)^T                           # [N, C]    (value)
    S  = f^T @ g                               # [N, N]
    O  = softmax(S, axis=-1) @ V               # [N, C]
    out[b] = O^T.reshape(C, H, W) + x[b]

Sharding: 8 cores = 4 batches x 2 query-halves; each core computes attention
for 2048 query positions against all 4096 keys. No cross-core communication.

Per-core pipeline (ACT-saturation schedule):
  - ACT(exp) is ~75us of unavoidable serial work (8.4M exps at ~1.2GHz,
    ACT is the only exp engine); PE matmul work is ~100us. The schedule
    therefore (1) starts the ACT stream as early as possible (first S^T
    round right after the first xkv chunk lands) and (2) never lets either
    engine idle: S^T rounds are emitted whenever the ACT queue runs low
    (~2 tiles), and the PE's remaining issue slots are backfilled with
    PV segments (4 m-tiles each) whose Pt slots are already exp'd.
  - f is computed REPLICATED on all four 32-partition strips; g with
    m-sub-tile t on strip t. S^T rounds run four K=32 matmuls concurrently
    via tile_position=(32t, 0) (~2x vs serial).
  - S^T PSUM tiles hold 1536 elems/partition -> one ACTIVATE(Exp) each
    (amortizes the ~450ns per-instruction ACT overhead); ACT writes Pt
    bf16 directly. A dummy fp32-PSUM exp during warmup hoists the ~2.7us
    ACT table load. (NB: a dummy with bf16-SBUF input leaves ACT ~20%
    slower for the whole run!)
  - V [4096, 257] bf16 with a ones column -> PV emits the softmax
    denominator free; O accumulated over 32 m-tiles in PSUM, in 8
    interleavable segments.
  - posts: r = 1/den; out = O*r + x (residual) in fp16, delayed by one
    segment group so DVE PSUM reads don't stall the PE stream.

Scheduling notes (hard-won):
  - PE clock (HAM) drops to 4/8 after ~1us idle and needs ~3.4us gapless
    streaming to recover: warmup starts immediately; dummy matmuls after
    the last PV segment hold the clock up through the output-DMA drain.
  - PSUM budget: S^T pool 2 x 3 banks + op pool (o_cur accumulators,
    f/g projections, warmup) 2 x 1 bank = 8 banks. V projections go in
    the S^T pool (interleaved with round tiles) because the op pool
    allows only ONE foreign allocation per o_cur window (the 2-deep
    rotation would otherwise deadlock the PE behind an accumulating o).
  - DMA: per-key-chunk transfers spread over the three DMA-capable rings
    (sync/scalar/gpsimd; one ring sustains only ~100GB/s) in need-order.
    Each DMA pays a ~3us completion-semaphore tax. xqt (residual) is
    DMA'd in per-chunk slices after the early xkv chunks so the first
    posts (~15us) don't wait on it.
"""

import sys

sys.path.insert(0, "/opt/trn_rl_repo")

from contextlib import ExitStack

import numpy as np

import concourse.bass as bass
import concourse.tile as tile
from concourse import bacc, mybir
from concourse.bass import ts, ds
from concourse.bass_utils import run_bass_kernel_spmd

F32 = mybir.dt.float32
F16 = mybir.dt.float16
BF16 = mybir.dt.bfloat16

B, C, H, W = 4, 256, 64, 64
N = H * W          # 4096 keys per batch
NQ = N // 2        # 2048 queries per core
CK = 32            # query/key head dim
MT = N // 128      # 32 m-tiles
NKC = N // 512     # 8 key chunks
EXP = mybir.ActivationFunctionType.Exp

CHUNK_W = 512
NCH = NQ // CHUNK_W            # 4 query chunks
SPT = 3                        # m-slots per S^T PSUM tile (3 banks)
NGRP = (MT + SPT - 1) // SPT   # 11 ACT groups per chunk
SEG = 4                        # m-tiles per PV segment
NSEG = MT // SEG               # 8 segments per (chunk, j)

# simulated issue costs (ns) for the static scheduler; fitted from traces
T_ROUND = 470.0
T_PV_SEG = SEG * 135.0
T_ACT_SLOT = 569.0
T_FPROJ = 740.0
T_G = 360.0
T_VHALF = 560.0
ACT_LAT = 250.0
ACT_AHEAD = 3400.0


def build_nc():
    nc = bacc.Bacc("TRN2", target_bir_lowering=False, debug=False, num_devices=8)
    xkv_d = nc.dram_tensor("xkv", [128, NKC, 2, 512], F16, kind="ExternalInput")
    xqt_d = nc.dram_tensor("xqt", [128, 16, 256], F16, kind="ExternalInput")
    wz_d = nc.dram_tensor("wz", [128, 640], F16, kind="ExternalInput")
    out_d = nc.dram_tensor("out", [128, 16, 256], F16, kind="ExternalOutput")

    with tile.TileContext(nc) as tc, ExitStack() as ctx:
        _body(ctx, tc, xkv_d.ap(), xqt_d.ap(), wz_d.ap(), out_d.ap())
    nc.compile()
    return nc


def _body(ctx, tc, xkv_d, xqt_d, wz_d, out_d):
    nc = tc.nc
    singles = ctx.enter_context(tc.tile_pool(name="singles", bufs=1))

    xkv = singles.tile([128, NKC, 2, 512], F16, tag="xkv", name="xkv")
    xqt = singles.tile([128, 16, 256], F16, tag="xqt", name="xqt")
    wz = singles.tile([128, 640], F16, tag="wz", name="wz")
    f4 = singles.tile([128, NQ], F16, tag="f4", name="f4")
    g4 = singles.tile([128, NKC, 128], F16, tag="g4", name="g4")
    V = singles.tile([128, MT, 260], BF16, tag="V", name="V")
    warm = singles.tile([128, 512], BF16, tag="warm", name="warm")
    # all allocations stay multiples of 64B/partition: a misaligned tile
    # here shifts Pt off 64B alignment and costs ~20% on ACT and PV LDWEIGHTS
    scr = singles.tile([128, 64], BF16, tag="scr", name="scr")

    # split so the first warm LDWEIGHTS (reads cols 0:128) fires before the
    # full-tile memset completes
    nc.vector.memset(warm[:, 0:128], 0.0)
    nc.vector.memset(warm[:, 128:512], 0.0)
    nc.gpsimd.memset(V[:, :, 256:257], 1.0)

    # PSUM: S^T pool 2 x 3 banks (also hosts V projections) + op pool
    # (o_cur, f/g projections, warmup) 2 x 1 bank = 8 banks.
    stp = ctx.enter_context(tc.tile_pool(name="st_ps", bufs=2, space="PSUM"))
    op = ctx.enter_context(tc.tile_pool(name="o_ps", bufs=2, space="PSUM"))
    ptp = ctx.enter_context(tc.tile_pool(name="pt", bufs=3))
    stgp = ctx.enter_context(tc.tile_pool(name="stage", bufs=2))
    osbp = ctx.enter_context(tc.tile_pool(name="osb", bufs=2))
    rp = ctx.enter_context(tc.tile_pool(name="r", bufs=2))

    w1t = [wz[:, 320 * k:320 * k + 32] for k in range(2)]
    w2t = [wz[:, 320 * k + 32:320 * k + 64] for k in range(2)]
    w3t = [wz[:, 320 * k + 64:320 * k + 320] for k in range(2)]

    # ---- input DMAs: chunks 0/1 split across both HW rings so the first
    # projections start ASAP; xqt per-chunk slices ride after early xkv ----
    nc.gpsimd.dma_start(wz[:], wz_d[:, :])
    for i in range(2):
        nc.sync.dma_start(xkv[:, i, 0, :], xkv_d[:, i, 0, :])
        nc.scalar.dma_start(xkv[:, i, 1, :], xkv_d[:, i, 1, :])
    for i, eng in [(2, nc.sync), (3, nc.scalar), (4, nc.gpsimd), (5, nc.sync),
                   (6, nc.scalar), (7, nc.gpsimd)]:
        eng.dma_start(xkv[:, i, :, :], xkv_d[:, i, :, :])
    # residual x^T: chunk ci needs xqt[:, 4ci:4ci+4, :] at its first post
    nc.sync.dma_start(xqt[:, 0:4, :], xqt_d[:, 0:4, :])
    nc.scalar.dma_start(xqt[:, 4:8, :], xqt_d[:, 4:8, :])
    nc.gpsimd.dma_start(xqt[:, 8:16, :], xqt_d[:, 8:16, :])

    # HAM warmup: keep the PE streaming while the first input DMAs land so
    # the clock gate opens before the projection phase.
    wps = [op.tile([128, 512], F32, tag="o", name="wps") for _ in range(2)]
    for i in range(14):
        nc.tensor.matmul(wps[i % 2][:], warm[:, 0:128], warm[:],
                         start=True, stop=True)

    # Dummy exp (same fp32-PSUM -> bf16-SBUF shape as the real calls) to
    # hoist the ~2.7us ACT table load into the warmup window.
    nc.scalar.activation(scr[:, 0:1], wps[0][:, 0:1], EXP)

    # ---- static scheduler state ----
    Pt = {}
    st_tiles = {}
    posts = []
    next_grp = [0] * NCH       # next un-emitted ACT group per chunk
    act_done = {}              # (ci, grp) -> simulated ACT completion time
    sim = {"pe": 0.0, "act": 0.0}

    def emit_post(item):
        ci, j, o_ps, stg = item
        J = 4 * ci + j
        r = rp.tile([128, 16], F32, tag="r", name="r")
        nc.vector.reciprocal(r[:, 0:1], o_ps[:, 256:257])
        o_sb = osbp.tile([128, 256], F16, tag="osb", name="osb")
        nc.vector.tensor_scalar_mul(o_sb[:], o_ps[:, 0:256], r[:, 0:1])
        nc.vector.tensor_add(stg[:, j, :], o_sb[:], xqt[:, J, :])
        if ci == NCH - 1:
            # stream the final chunk's output per j-tile so the last DMA
            # (and its ~3us completion tax) starts as early as possible
            engs = [nc.sync, nc.gpsimd, nc.scalar]
            engs[j % 3].dma_start(out_d[:, J:J + 1, :], stg[:, j:j + 1, :])
        elif j == 3:
            nc.gpsimd.dma_start(out_d[:, 4 * ci:4 * ci + 4, :], stg[:, 0:4, :])

    def st_round(ci, r):
        # Row-tiled round: 4 concurrent K=32 matmuls for m-tiles 4r..4r+3 of
        # query chunk ci; strip t holds g for m-sub-tile t of key chunk r.
        for t in range(4):
            s = 4 * r + t
            q, sub = divmod(s, SPT)
            if sub == 0:
                st_tiles[(ci, q)] = stp.tile([128, SPT, CHUNK_W], F32,
                                             tag="st", name="st")
            nc.tensor.matmul(st_tiles[(ci, q)][:, sub, :],
                             g4[32 * t:32 * t + 32, r, :],
                             f4[32 * t:32 * t + 32, ds(CHUNK_W * ci, CHUNK_W)],
                             start=True, stop=True, tile_position=(32 * t, 0),
                             skip_group_check=True)
        sim["pe"] += T_ROUND
        # emit ACTs for groups fully covered by m-tiles <= 4r+3
        while next_grp[ci] < NGRP:
            q = next_grp[ci]
            nslot = min(SPT, MT - SPT * q)
            if SPT * q + nslot - 1 > 4 * r + 3:
                break
            tl = st_tiles.pop((ci, q))
            nc.scalar.activation(Pt[ci][:, SPT * q:SPT * q + nslot, :],
                                 tl[:, 0:nslot, :], EXP)
            sim["act"] = max(sim["act"], sim["pe"] + ACT_LAT) + T_ACT_SLOT * nslot
            act_done[(ci, q)] = sim["act"]
            next_grp[ci] += 1

    # NB: all projection PSUM tiles live in the S^T pool (tag "st"): the op
    # pool only ever rotates o_cur accumulators (plus the two warmup tiles),
    # so a projection emitted mid-o-window can never park the PE behind a
    # post that hasn't been emitted yet (2-deep rotation deadlock).
    def fproj(qc):
        fp = stp.tile([128, 512], F32, tag="st", name="fp")
        for k in range(2):
            for j in range(4):
                nc.tensor.matmul(fp[32 * j:32 * j + 32, :], w1t[k],
                                 xkv[:, qc, k, :], start=(k == 0),
                                 stop=(k == 1), tile_position=(0, 32 * j),
                                 skip_group_check=True)
        nc.vector.tensor_copy(f4[:, ts(qc, 512)], fp[:])
        sim["pe"] += T_FPROJ

    def gproj(i):
        gp = stp.tile([128, 128], F32, tag="st", name="gp")
        for k in range(2):
            for t in range(4):
                nc.tensor.matmul(gp[32 * t:32 * t + 32, :], w2t[k],
                                 xkv[:, i, k, ts(t, 128)], start=(k == 0),
                                 stop=(k == 1), tile_position=(0, 32 * t),
                                 skip_group_check=True)
        nc.vector.tensor_copy(g4[:, i, :], gp[:])
        sim["pe"] += T_G

    def vproj_half(i, p):
        # V for n-tiles 4i+2p .. 4i+2p+1
        vp = stp.tile([128, 2, 256], F32, tag="st", name="vp")
        for u in range(2):
            for k in range(2):
                nc.tensor.matmul(vp[:, u, :], xkv[:, i, k, ts(2 * p + u, 128)],
                                 w3t[k], start=(k == 0), stop=(k == 1))
        nc.vector.tensor_copy(V[:, 4 * i + 2 * p:4 * i + 2 * p + 2, 0:256],
                              vp[:])
        sim["pe"] += T_VHALF

    # estimated xkv chunk DMA arrival times (ns) for emission pacing only;
    # real ordering is enforced by Tile-framework semaphores
    xkv_ready = [3800, 5000, 7600, 7600, 6700, 10200, 10200, 9200]

    def gate(t):
        sim["pe"] = max(sim["pe"], t)

    # ---- phase 1: projections for early chunks + first rounds, so the ACT
    # stream opens as soon as xkv chunk 0 lands ----
    sim["pe"] = 5400.0 + 14 * 241.0  # engine init + warmup
    Pt[0] = ptp.tile([128, MT, CHUNK_W], BF16, tag="pt", name="pt")
    gate(xkv_ready[0])
    fproj(0)
    gproj(0)
    st_round(0, 0)
    vproj_half(0, 0)
    vproj_half(0, 1)
    gate(xkv_ready[1])
    fproj(1)
    gproj(1)
    st_round(0, 1)
    vproj_half(1, 0)
    vproj_half(1, 1)

    # remaining work queues
    rounds_q = [(ci, r) for ci in range(NCH) for r in range(NKC)][2:]
    # proj items still pending, in need-order
    proj_q = []
    for i in range(2, NKC):
        proj_q.append(("g", i))
        proj_q.append(("vh", i, 0))
        proj_q.append(("vh", i, 1))
    proj_q.insert(3, ("f", 2))
    proj_q.insert(7, ("f", 3))
    pv_q = [(ci, j, s) for ci in range(NCH) for j in range(4)
            for s in range(NSEG)]
    g_emitted = {0, 1}
    f_emitted = {0, 1}

    cur_o = {}   # (ci, j) -> psum tile, for the active o accumulator
    cur_stg = {}

    def emit_proj(item, stall=False):
        if stall:
            gate(xkv_ready[item[1]])
        if item[0] == "g":
            gproj(item[1])
            g_emitted.add(item[1])
        elif item[0] == "f":
            fproj(item[1])
            f_emitted.add(item[1])
        else:
            vproj_half(item[1], item[2])

    def proj_data_ready(item):
        return sim["pe"] >= xkv_ready[item[1]] - 300.0

    def round_ready(rc):
        ci, r = rc
        return ci in f_emitted and r in g_emitted

    def pv_seg(ci, j, s):
        if s == 0:
            cur_o[(ci, j)] = op.tile([128, 257], F32, tag="o", name="o")
            if j == 0:
                cur_stg[ci] = stgp.tile([128, 4, 256], F16, tag="stage",
                                        name="stage")
        o_cur = cur_o[(ci, j)]
        for mm in range(SEG):
            mt = SEG * s + mm
            nc.tensor.matmul(o_cur[:], Pt[ci][:, mt, ts(j, 128)],
                             V[:, mt, 0:257],
                             start=(mt == 0), stop=(mt == MT - 1),
                             skip_group_check=True)
        sim["pe"] += T_PV_SEG
        if s == NSEG - 1:
            posts.append((ci, j, o_cur, cur_stg[ci]))
            last = ci == NCH - 1 and j == 3
            while len(posts) > (0 if last else 1):
                emit_post(posts.pop(0))

    def pv_ready(pv):
        ci, j, s = pv
        g0 = (SEG * s) // SPT
        g1 = (SEG * s + SEG - 1) // SPT
        for q in range(g0, g1 + 1):
            t = act_done.get((ci, q))
            if t is None or t > sim["pe"] + 120.0:
                return False
        return True

    def pop_round(stall=False):
        ci, r = rounds_q.pop(0)
        if r == 0 and ci not in Pt:
            Pt[ci] = ptp.tile([128, MT, CHUNK_W], BF16, tag="pt", name="pt")
        if stall:
            # PE parks at the S^T pool semaphore until ACT drains a tile
            gate(sim["act"] - T_ACT_SLOT * SPT)
        st_round(ci, r)

    # ---- phase 2: unified emission loop ----
    while rounds_q or proj_q or pv_q:
        if (rounds_q and round_ready(rounds_q[0])
                and sim["act"] - sim["pe"] < ACT_AHEAD):
            pop_round()
            continue
        if proj_q and proj_data_ready(proj_q[0]):
            emit_proj(proj_q.pop(0))
            continue
        if pv_q and pv_ready(pv_q[0]):
            ci, j, s = pv_q.pop(0)
            pv_seg(ci, j, s)
            continue
        # nothing cleanly ready: prefer feeding ACT (round, stalling at the
        # S^T pool) over a proj stalled on DMA over a PV stalled on ACT
        if rounds_q and round_ready(rounds_q[0]):
            pop_round(stall=True)
            continue
        if proj_q:
            emit_proj(proj_q.pop(0), stall=True)
            continue
        if pv_q:
            ci, j, s = pv_q.pop(0)
            g1 = (SEG * s + SEG - 1) // SPT
            t = act_done.get((ci, g1))
            if t is not None:
                sim["pe"] = max(sim["pe"], t)
            pv_seg(ci, j, s)
            continue

    while posts:
        emit_post(posts.pop(0))

    # hold the HAM clock up through the post/DMA drain so the epilogue
    # barrier doesn't run at half clock
    for i in range(20):
        tl = stp.tile([128, SPT, CHUNK_W], F32, tag="st", name="st")
        nc.tensor.matmul(tl[:, 0, :], warm[:, 0:128], warm[:],
                         start=True, stop=True)


_NC_CACHE = None


def _get_nc():
    global _NC_CACHE
    if _NC_CACHE is None:
        _NC_CACHE = build_nc()
    return _NC_CACHE


def make_in_maps(x, w1, w2, w3):
    x = np.ascontiguousarray(x, dtype=np.float32).reshape(B, C, N)
    xh = x.astype(np.float16)
    # weights packed [128, 640]: per k-half, cols 0:32 w1^T, 32:64 w2^T,
    # 64:320 w3^T (k=0 at 0:320, k=1 at 320:640) -> one contiguous DMA run
    wz = np.empty((128, 640), dtype=np.float16)
    for k in range(2):
        wz[:, 320 * k:320 * k + 32] = w1.T[128 * k:128 * k + 128, :]
        wz[:, 320 * k + 32:320 * k + 64] = w2.T[128 * k:128 * k + 128, :]
        wz[:, 320 * k + 64:320 * k + 320] = w3.T[128 * k:128 * k + 128, :]
    in_maps = []
    for core in range(8):
        b, half = core // 2, core % 2
        # key-chunk permutation: this core's query chunks first
        perm = [4 * half + i for i in range(4)] + \
               [4 * (1 - half) + i for i in range(4)]
        # xkv packed [128, 8, 2, 512]: [p, ch, k, s] = xh[b][128k+p, 512*perm[ch]+s]
        xv = xh[b].reshape(2, 128, NKC, 512).transpose(1, 2, 0, 3)[:, perm]
        # residual x^T for this core's queries: [2048, 256] -> [128, 16, 256]
        xq = xh[b][:, half * NQ:(half + 1) * NQ].T
        xqt = xq.reshape(16, 128, 256).transpose(1, 0, 2)
        in_maps.append({
            "xkv": np.ascontiguousarray(xv),
            "xqt": np.ascontiguousarray(xqt),
            "wz": wz,
        })
    return in_maps


def assemble(results):
    out = np.empty((B, C, N), dtype=np.float32)
    for core in range(8):
        b, half = core // 2, core % 2
        o = results[core]["out"].astype(np.float32)      # [128, 16, 256]
        o = o.transpose(1, 0, 2).reshape(NQ, C)          # [(J p), c]
        out[b][:, half * NQ:(half + 1) * NQ] = o.T
    return out.reshape(B, C, H, W)


def kernel(x, w1, w2, w3):
    nc = _get_nc()
    res = run_bass_kernel_spmd(nc, make_in_maps(x, w1, w2, w3),
                               core_ids=list(range(8)))
    return assemble(res.results)
